# revision 53
# baseline (speedup 1.0000x reference)
"""Trainium2 Bass kernel: single transformer block (MHA + FFN + 2xLN).

Sharding: data-parallel over tokens. 8 cores; cores 0-3 own batch 0,
cores 4-7 own batch 1; each core owns 1024 consecutive tokens of its
batch. QKV/FFN/LN are token-local; attention needs all K/V of the
batch, obtained with pipelined AllGathers over each 4-core group
(3 K-gathers launched during the K GEMM + 3 V-gathers).

Key layout decisions:
- All weight transposes/layout shuffles are done on the HOST in numpy
  (free): weights arrive as bf16 lhsT tiles in exact SBUF layout, x
  arrives pre-transposed feature-major, and the output is returned
  feature-major and untransposed on the host.
- All GEMMs run bf16 operands with fp32 PSUM accumulation.
- Scores use 2-head row-tiling: heads 2m/2m+1 live on partitions
  0-63/64-127 of the same K^T/Q^T chunk, so their K=64 matmuls run
  concurrently on different row-groups of the PE array.
- Softmax skips the max subtraction (scores are O(6)); the denominator
  falls out of the ctx matmul via a per-head ones column in V.
- LayerNorm runs along the partition (feature) axis with ones-vector
  matmul sums and PE row-broadcasts.
"""

import os
import sys

for _p in (
    "/opt/trn_rl_repo",
    "/root/.axon_site",
    "/root/.axon_site/_ro/trn_rl_repo",
    "/root/.axon_site/_ro/pypackages",
):
    if os.path.isdir(_p) and _p not in sys.path:
        sys.path.append(_p)

import ml_dtypes
import numpy as np

import concourse.bass as bass  # noqa: F401  (import keeps bass registered)
import concourse.mybir as mybir
import concourse.tile as tile
from concourse import bacc
from concourse.bass_utils import run_bass_kernel_spmd

F32 = mybir.dt.float32
F32R = mybir.dt.float32r
BF16 = mybir.dt.bfloat16
NPBF16 = ml_dtypes.bfloat16
AF = mybir.ActivationFunctionType
ALU = mybir.AluOpType

B, S, D = 2, 4096, 768
H, DK = 12, 64
DFF = 3072
NCORES = 8
GROUP = 4  # cores per batch
TOK = (B * S) // NCORES  # 1024 tokens per core
TCH = TOK // 128  # 8
DCH = D // 128  # 6
FCH = DFF // 128  # 24
KV = S  # kv length per batch
KCH = KV // 128  # 32
EPS = 1e-5
RG = [[0, 1, 2, 3], [4, 5, 6, 7]]

NG = 3  # head groups (sub-gathers); 4 heads each
HPG = H // NG
VWG = HPG * 65  # 260: V columns per group incl. per-head ones column
KREG = 128 * 2 * TOK  # bf16 elems of K^T per K sub-gather (2 chunks)
VREG = TCH * 128 * VWG  # bf16 elems of V per V sub-gather


def _emit_ln(tc, ps_bc, ps_st, y, g_sb, beta_sb, out, tmp_pool,
             out_dram=None):
    """LayerNorm along the partition (feature) axis of y [128, 6*1024]."""
    nc = tc.nc
    ones_pb, ones_fr = tc._ones_pb, tc._ones_fr
    st = ps_st.tile([1, TOK], F32, tag="st", name="st")
    stq = ps_st.tile([1, TOK], F32, tag="stq", name="stq")
    for j in range(DCH):
        js = slice(j * TOK, (j + 1) * TOK)
        sq = tmp_pool.tile([128, TOK], BF16, tag="lnsq", name="sq")
        nc.vector.tensor_tensor(sq[:], y[:, js], y[:, js], ALU.mult)
        for q in range(2):
            qs = slice(q * 512, (q + 1) * 512)
            jq = slice(j * TOK + q * 512, j * TOK + (q + 1) * 512)
            nc.tensor.matmul(st[0:1, qs], ones_pb[:], y[:, jq],
                             start=(j == 0), stop=(j == DCH - 1),
                             skip_group_check=True)
            nc.tensor.matmul(stq[0:1, qs], ones_pb[:], sq[:, qs],
                             start=(j == 0), stop=(j == DCH - 1),
                             skip_group_check=True)
    mu = tmp_pool.tile([1, TOK], F32, tag="lnmu", name="mu")
    var = tmp_pool.tile([1, TOK], F32, tag="lnvar", name="var")
    mm = tmp_pool.tile([1, TOK], F32, tag="lnmm", name="mm")
    rs = tmp_pool.tile([1, TOK], F32, tag="lnrs", name="rs")
    rsr = tmp_pool.tile([1, TOK], F32R, tag="lnrsr", name="rsr")
    mur = tmp_pool.tile([1, TOK], F32R, tag="lnmur", name="mur")
    nc.scalar.activation(mu[:], st[0:1, :], AF.Copy, scale=1.0 / D)
    nc.scalar.activation(var[:], stq[0:1, :], AF.Copy, scale=1.0 / D)
    nc.vector.tensor_tensor(mm[:], mu[:], mu[:], ALU.mult)
    nc.vector.tensor_tensor(var[:], var[:], mm[:], ALU.subtract)
    nc.scalar.activation(var[:], var[:], AF.Sqrt, bias=tc._eps[:])
    nc.vector.reciprocal_approx_fast(rs[:], var[:])
    nc.vector.tensor_copy(rsr[:], rs[:])
    nc.vector.tensor_tensor(mur[:], mu[:], rsr[:], ALU.mult)
    bcA = ps_bc.tile([128, TOK], F32, tag="bc", name="bcA")
    bcB = ps_bc.tile([128, TOK], F32, tag="bc", name="bcB")
    for q in range(2):
        qs = slice(q * 512, (q + 1) * 512)
        nc.tensor.matmul(bcA[:, qs], ones_fr[:], rsr[:, qs],
                         start=True, stop=True, skip_group_check=True)
        nc.tensor.matmul(bcB[:, qs], ones_fr[:], mur[:, qs],
                         start=True, stop=True, skip_group_check=True)
    for j in range(DCH):
        js = slice(j * TOK, (j + 1) * TOK)
        t1 = tmp_pool.tile([128, TOK], F32, tag="lnt", name="t1")
        nc.vector.tensor_tensor(t1[:], y[:, js], bcA[:], ALU.mult)
        nc.vector.tensor_tensor(t1[:], t1[:], bcB[:], ALU.subtract)
        if out_dram is None:
            nc.vector.tensor_scalar(out[:, js], t1[:],
                                    g_sb[:, j : j + 1], beta_sb[:, j : j + 1],
                                    ALU.mult, ALU.add)
        else:
            ot = tmp_pool.tile([128, TOK], F32, tag="lnot", name="ot")
            nc.vector.tensor_scalar(ot[:], t1[:],
                                    g_sb[:, j : j + 1], beta_sb[:, j : j + 1],
                                    ALU.mult, ALU.add)
            nc.sync.dma_start(out_dram[:, js], ot[:])


def _emit_body(tc, t_in, t_out):
    nc = tc.nc
    out_ap = t_out["out_shard"]

    with tc.tile_pool(name="const", bufs=1) as const, \
         tc.tile_pool(name="dram", bufs=1, space="DRAM") as dram, \
         tc.tile_pool(name="pMain", bufs=1) as pMain:

        # ---- constants ----
        ones_f32 = const.tile([1, 128], F32)
        nc.vector.memset(ones_f32[:], 1.0)
        ones_fr = const.tile([1, 128], F32R)
        nc.vector.tensor_copy(ones_fr[:], ones_f32[:])
        ones_1b = const.tile([1, 128], BF16)
        nc.vector.tensor_copy(ones_1b[:], ones_f32[:])
        ones_pb = const.tile([128, 1], BF16)
        nc.vector.memset(ones_pb[:], 1.0)
        ones_col12 = const.tile([128, H], BF16)
        nc.vector.memset(ones_col12[:], 1.0)
        eps_sb = const.tile([1, 1], F32)
        nc.vector.memset(eps_sb[:], EPS)
        tc._ones_fr, tc._ones_pb, tc._eps = ones_fr, ones_pb, eps_sb

        # ---- DRAM scratch for collectives ----
        k_ins = [dram.tile([KREG], BF16, tag=f"ki{g}", name=f"k_in{g}")
                 for g in range(NG)]
        k_outs = [dram.tile([GROUP, KREG], BF16, tag=f"ko{g}", name=f"k_out{g}")
                  for g in range(NG)]
        v_ins = [dram.tile([VREG], BF16, tag=f"vi{g}", name=f"v_in{g}")
                 for g in range(NG)]
        v_outs = [dram.tile([GROUP, VREG], BF16, tag=f"vo{g}", name=f"v_out{g}")
                  for g in range(NG)]

        # ---- persistent SBUF tiles ----
        xT = pMain.tile([128, DCH * TOK], BF16, tag="slotX", name="xT")
        QT = pMain.tile([128, DCH * TOK], BF16, tag="slotQ", name="QT")
        woT = pMain.tile([128, DCH * D], BF16, tag="woT", name="woT")
        bv_row = pMain.tile([1, D], BF16, tag="bv", name="bv")

        def load_pcol(name, n):
            t = pMain.tile([128, n], F32, tag=f"pc_{name}", name=f"pc_{name}")
            nc.sync.dma_start(t[:], t_in[name])
            return t

        # ---- Phase A: QKV projections + scatter/gather launches ----
        with tc.tile_pool(name="pA", bufs=1) as pA, \
             tc.tile_pool(name="psA", bufs=2, space="PSUM") as psA:
            wkT = pA.tile([128, DCH * D], BF16, tag="wkT", name="wkT")
            wvT = pA.tile([128, DCH * D], BF16, tag="wvT", name="wvT")
            wqT = pA.tile([128, DCH * D], BF16, tag="wqT", name="wqT")
            KT = pMain.tile([128, DCH * TOK], BF16, tag="slotC", name="KT")
            vt = pA.tile([128, TCH * (H * 65)], BF16, tag="vt", name="vt")

            # critical-path loads spread across the sync + scalar DGE queues
            nc.scalar.dma_start(wkT[:], t_in["wkT"])
            nc.sync.dma_start(xT[:], t_in["xT"])
            bk_sb = load_pcol("bk", DCH)
            nc.scalar.dma_start(wvT[:], t_in["wvT"])
            nc.sync.dma_start(bv_row[:], t_in["bv"])
            nc.scalar.dma_start(wqT[:], t_in["wqT"])
            bq_sb = load_pcol("bq", DCH)
            bo_sb = load_pcol("bo", DCH)
            b1_sb = load_pcol("b1", FCH)
            b2_sb = load_pcol("b2", DCH)
            g1_sb = load_pcol("g1", DCH)
            beta1_sb = load_pcol("beta1", DCH)
            g2_sb = load_pcol("g2", DCH)
            beta2_sb = load_pcol("beta2", DCH)
            # off-critical-path prefetch on the scalar DGE queue (gpsimd's
            # queue carries the collectives; sync carries the hot loads)
            nc.scalar.dma_start(woT[:], t_in["woT"])

            def proj(wT, b_sb, dest):
                for m in range(DCH):
                    pso = psA.tile([128, TOK], F32, tag="qk", name="pso")
                    for q in range(2):
                        qs = slice(q * 512, (q + 1) * 512)
                        for j in range(DCH):
                            nc.tensor.matmul(
                                pso[:, qs],
                                wT[:, j * D + m * 128 : j * D + (m + 1) * 128],
                                xT[:, j * TOK + q * 512 : j * TOK + (q + 1) * 512],
                                start=(j == 0), stop=(j == DCH - 1),
                                skip_group_check=True)
                    nc.vector.tensor_scalar_add(
                        dest[:, m * TOK : (m + 1) * TOK], pso[:],
                        b_sb[:, m : m + 1])
                    yield m

            # K first: scatter each chunk; only K sub-gather 0 launches here
            # (gathers 1/2 are interleaved with the V sub-gathers below)
            for m in proj(wkT, bk_sb, KT):
                g, c = divmod(m, 2)
                nc.sync.dma_start(
                    k_ins[g][c * 128 * TOK : (c + 1) * 128 * TOK].rearrange(
                        "(p t) -> p t", p=128),
                    KT[:, m * TOK : (m + 1) * TOK])
                if m == 1:
                    nc.gpsimd.collective_compute(
                        "AllGather", ALU.bypass, replica_groups=RG,
                        ins=[k_ins[0][:].opt()], outs=[k_outs[0][:].opt()])

            # V in natural [tok, feat] layout with per-head ones columns,
            # one head-group at a time so each V sub-gather launches early.
            # Gather launch order interleaves K and V (Kg0 already queued in
            # the K loop): Vg0, Kg1, Kg2 were held back so the CC stream
            # serves Vg0 right after Kg0 (ctx of pair 0 needs it early).
            for g in range(NG):
                for t in range(TCH):
                    psv = psA.tile([128, 4 * DK], F32, tag="v", name="psv")
                    for j in range(DCH):
                        nc.tensor.matmul(
                            psv[:],
                            xT[:, j * TOK + t * 128 : j * TOK + (t + 1) * 128],
                            wvT[:, j * D + g * 256 : j * D + (g + 1) * 256],
                            start=(j == 0), stop=False, skip_group_check=True)
                    nc.tensor.matmul(
                        psv[:], ones_1b[:],
                        bv_row[:, g * 256 : (g + 1) * 256],
                        start=False, stop=True, skip_group_check=True)
                    vts = vt[:, t * (H * 65) + g * VWG :
                             t * (H * 65) + (g + 1) * VWG]
                    vtr = vts.rearrange("p (h f) -> p h f", h=HPG)
                    nc.vector.tensor_copy(
                        vtr[:, :, 0:DK],
                        psv[:].rearrange("p (h f) -> p h f", h=HPG))
                    nc.vector.tensor_copy(vtr[:, :, DK : DK + 1],
                                          ones_col12[:, 0:HPG].unsqueeze(2))
                    nc.sync.dma_start(
                        v_ins[g][t * 128 * VWG : (t + 1) * 128 * VWG]
                        .rearrange("(p f) -> p f", p=128),
                        vts[:])
                nc.gpsimd.collective_compute(
                    "AllGather", ALU.bypass, replica_groups=RG,
                    ins=[v_ins[g][:].opt()], outs=[v_outs[g][:].opt()])
                if g < 2:  # release the held-back K sub-gather g+1
                    nc.gpsimd.collective_compute(
                        "AllGather", ALU.bypass, replica_groups=RG,
                        ins=[k_ins[g + 1][:].opt()],
                        outs=[k_outs[g + 1][:].opt()])

            # Q last (only needed once attention starts)
            for _ in proj(wqT, bq_sb, QT):
                pass

        # ---- Phase B: attention (+ FFN weight prefetch into pA's space) ----
        ctxT = pMain.tile([128, DCH * TOK], BF16, tag="slotC", name="ctxT")
        w1T = pMain.tile([128, DCH * DFF], BF16, tag="w1T", name="w1T")
        w2T = pMain.tile([128, FCH * D], BF16, tag="w2T", name="w2T")
        nc.scalar.dma_start(w1T[:], t_in["w1T"])
        nc.scalar.dma_start(w2T[:], t_in["w2T"])
        with tc.tile_pool(name="pKG", bufs=2) as pKG, \
             tc.tile_pool(name="pVG", bufs=1) as pVG, \
             tc.tile_pool(name="pE", bufs=6) as pE, \
             tc.tile_pool(name="pB", bufs=1) as pB, \
             tc.tile_pool(name="psBs", bufs=2, space="PSUM") as psBs, \
             tc.tile_pool(name="psB", bufs=1, space="PSUM") as psB:
            for g in range(NG):
                KG = pKG.tile([128, 2 * KV], BF16, tag="KG", name=f"KG{g}")
                VG = pVG.tile([128, KCH * VWG], BF16, tag="VG", name=f"VG{g}")
                for r in range(GROUP):
                    for c in range(2):
                        nc.sync.dma_start(
                            KG[:, c * KV + r * TOK : c * KV + (r + 1) * TOK],
                            k_outs[g][r, c * 128 * TOK : (c + 1) * 128 * TOK]
                            .rearrange("(p t) -> p t", p=128))
                    nc.sync.dma_start(
                        VG[:, r * TCH * VWG : (r + 1) * TCH * VWG].rearrange(
                            "p (t f) -> p t f", t=TCH),
                        v_outs[g][r].rearrange("(t p f) -> p t f", t=TCH,
                                               p=128))
                for pl in range(2):
                    m = 2 * g + pl
                    ctxa = psB.tile([128, TOK], F32, tag="ctxa", name="ctxa")
                    ctxb = psB.tile([128, TOK], F32, tag="ctxb", name="ctxb")
                    for c in range(KCH):
                        # Per q-half, one PSUM tile holds BOTH heads' scores
                        # ([a | b] side by side): the row-tiled a/b matmuls
                        # are adjacent (LDW pull-ahead across row groups) and
                        # one exp covers both heads' half.
                        kof = pl * KV + c * 128
                        vof = c * VWG + 2 * pl * 65
                        Es = []
                        for q in range(2):
                            ps = psBs.tile([128, TOK], F32, tag="s", name="ps")
                            qof = slice(m * TOK + q * 512,
                                        m * TOK + (q + 1) * 512)
                            nc.tensor.matmul(
                                ps[:, 0:512], KG[0:64, kof : kof + 128],
                                QT[0:64, qof], start=True, stop=True,
                                skip_group_check=True)
                            nc.tensor.matmul(
                                ps[:, 512:1024], KG[64:128, kof : kof + 128],
                                QT[64:128, qof], start=True, stop=True,
                                skip_group_check=True)
                            E = pE.tile([128, TOK], BF16, tag="E", name="E")
                            nc.scalar.activation(E[:], ps[:], AF.Exp,
                                                 scale=0.125)
                            Es.append(E)
                        for q in range(2):
                            qs = slice(q * 512, (q + 1) * 512)
                            nc.tensor.matmul(
                                ctxa[0:65, qs], VG[:, vof : vof + 65],
                                Es[q][:, 0:512], start=(c == 0),
                                stop=(c == KCH - 1), skip_group_check=True)
                            nc.tensor.matmul(
                                ctxb[0:65, qs], VG[:, vof + 65 : vof + 130],
                                Es[q][:, 512:1024], start=(c == 0),
                                stop=(c == KCH - 1), skip_group_check=True)
                    # evict raw (unnormalized) ctx + its denominator row so
                    # the PSUM accumulators free immediately (no PE stall at
                    # the pair boundary), then normalize lazily off-path.
                    for hh, ctx_ps, ctag in ((0, ctxa, "ctxa"),
                                             (1, ctxb, "ctxb")):
                        dd = pB.tile([1, TOK], F32, tag="dd", name="dd")
                        rec = pB.tile([1, TOK], F32, tag="rec", name="rec")
                        recr = pB.tile([1, TOK], F32R, tag="recr",
                                       name="recr")
                        nc.vector.tensor_copy(dd[:], ctx_ps[64:65, :])
                        cs = ctxT[hh * 64 : hh * 64 + 64,
                                  m * TOK : (m + 1) * TOK]
                        nc.vector.tensor_copy(cs, ctx_ps[0:64, :])
                        nc.vector.reciprocal_approx_fast(rec[:], dd[:])
                        nc.vector.tensor_copy(recr[:], rec[:])
                        # broadcast rides the just-freed ctx PSUM ring, NOT
                        # the scores ring: the next pair's scores must never
                        # wait on this chain (keeps the PE gap-free)
                        bc = psB.tile([128, TOK], F32, tag=ctag, name="bc")
                        for q in range(2):
                            qs = slice(q * 512, (q + 1) * 512)
                            nc.tensor.matmul(bc[:, qs], ones_fr[:],
                                             recr[0:1, qs], start=True,
                                             stop=True,
                                             skip_group_check=True)
                        nc.vector.tensor_tensor(cs, cs, bc[0:64, :],
                                                ALU.mult)

        # ---- Phase C: O-projection + residual + LN1 ----
        y1 = pMain.tile([128, DCH * TOK], BF16, tag="slotQ", name="y1")
        n1 = pMain.tile([128, DCH * TOK], BF16, tag="slotN", name="n1")
        with tc.tile_pool(name="pC", bufs=2) as pC, \
             tc.tile_pool(name="psC", bufs=2, space="PSUM") as psC, \
             tc.tile_pool(name="psCs", bufs=1, space="PSUM") as psCs:
            for m in range(DCH):
                pso = psC.tile([128, TOK], F32, tag="bc", name="pso")
                for q in range(2):
                    qs = slice(q * 512, (q + 1) * 512)
                    for j in range(DCH):
                        nc.tensor.matmul(
                            pso[:, qs],
                            woT[:, j * D + m * 128 : j * D + (m + 1) * 128],
                            ctxT[:, j * TOK + q * 512 : j * TOK + (q + 1) * 512],
                            start=(j == 0), stop=(j == DCH - 1),
                            skip_group_check=True)
                nc.vector.scalar_tensor_tensor(
                    y1[:, m * TOK : (m + 1) * TOK], pso[:],
                    bo_sb[:, m : m + 1], xT[:, m * TOK : (m + 1) * TOK],
                    ALU.add, ALU.add)
            _emit_ln(tc, psC, psCs, y1, g1_sb, beta1_sb, n1, pC)

        # ---- Phase D: FFN (+ residual) ----
        y2 = pMain.tile([128, DCH * TOK], BF16, tag="slotX", name="y2")
        with tc.tile_pool(name="pD", bufs=3) as pD, \
             tc.tile_pool(name="psD", bufs=1, space="PSUM") as psD, \
             tc.tile_pool(name="psDh", bufs=2, space="PSUM") as psDh:
            for half in range(2):
                ps2 = psD.tile([128, DCH * 512], F32, tag="ffn2", name="ps2")
                for i in range(FCH):
                    psh = psDh.tile([128, 512], F32, tag="h", name="psh")
                    for j in range(DCH):
                        nc.tensor.matmul(
                            psh[:],
                            w1T[:, j * DFF + i * 128 : j * DFF + (i + 1) * 128],
                            n1[:, j * TOK + half * 512 :
                               j * TOK + (half + 1) * 512],
                            start=(j == 0), stop=(j == DCH - 1),
                            skip_group_check=True)
                    hsb = pD.tile([128, 512], BF16, tag="hsb", name="hsb")
                    nc.scalar.activation(hsb[:], psh[:], AF.Gelu,
                                         bias=b1_sb[:, i : i + 1])
                    for mm in range(DCH):
                        nc.tensor.matmul(
                            ps2[:, mm * 512 : (mm + 1) * 512],
                            w2T[:, i * D + mm * 128 : i * D + (mm + 1) * 128],
                            hsb[:], start=(i == 0), stop=(i == FCH - 1),
                            skip_group_check=True)
                for mm in range(DCH):
                    nc.vector.scalar_tensor_tensor(
                        y2[:, mm * TOK + half * 512 :
                           mm * TOK + (half + 1) * 512],
                        ps2[:, mm * 512 : (mm + 1) * 512],
                        b2_sb[:, mm : mm + 1],
                        n1[:, mm * TOK + half * 512 :
                           mm * TOK + (half + 1) * 512],
                        ALU.add, ALU.add)

        # ---- Phase E: LN2 + output ----
        with tc.tile_pool(name="pEo", bufs=2) as pEo, \
             tc.tile_pool(name="psE", bufs=2, space="PSUM") as psE, \
             tc.tile_pool(name="psEs", bufs=1, space="PSUM") as psEs:
            _emit_ln(tc, psE, psEs, y2, g2_sb, beta2_sb, None, pEo,
                     out_dram=out_ap)


_CACHE = {}


def _build():
    if "nc" in _CACHE:
        return _CACHE["nc"]
    nc = bacc.Bacc("TRN2", target_bir_lowering=False, debug=False,
                   num_devices=NCORES)
    t_in = {}
    for name, shape, dt in (
        ("xT", [128, DCH * TOK], BF16),
        ("wqT", [128, DCH * D], BF16),
        ("wkT", [128, DCH * D], BF16),
        ("wvT", [128, DCH * D], BF16),
        ("woT", [128, DCH * D], BF16),
        ("w1T", [128, DCH * DFF], BF16),
        ("w2T", [128, FCH * D], BF16),
        ("bv", [1, D], BF16),
        ("bq", [128, DCH], F32),
        ("bk", [128, DCH], F32),
        ("bo", [128, DCH], F32),
        ("b1", [128, FCH], F32),
        ("b2", [128, DCH], F32),
        ("g1", [128, DCH], F32),
        ("beta1", [128, DCH], F32),
        ("g2", [128, DCH], F32),
        ("beta2", [128, DCH], F32),
    ):
        t_in[name] = nc.dram_tensor(name, shape, dt, kind="ExternalInput").ap()
    t_out = {"out_shard": nc.dram_tensor("out_shard", [128, DCH * TOK], F32,
                                         kind="ExternalOutput").ap()}
    with tile.TileContext(nc) as tc:
        _emit_body(tc, t_in, t_out)
    nc.compile()
    _CACHE["nc"] = nc
    return nc


def _wT_layout(w, nin_ch):
    """torch-Linear weight [out, in] -> lhsT SBUF layout [128, nin_ch*out]."""
    w = np.asarray(w, np.float32)
    nout = w.shape[0]
    t = w.T.reshape(nin_ch, 128, nout).transpose(1, 0, 2)
    return np.ascontiguousarray(t).astype(NPBF16).reshape(128, nin_ch * nout)


def _pcol(b, n):
    return np.ascontiguousarray(np.asarray(b, np.float32).reshape(n, 128).T)


def _in_maps(inputs):
    f32 = lambda k: np.asarray(inputs[k], np.float32)
    shared = {
        "wqT": _wT_layout(inputs["wq"], DCH),
        "wkT": _wT_layout(inputs["wk"], DCH),
        "wvT": _wT_layout(inputs["wv"], DCH),
        "woT": _wT_layout(inputs["wo"], DCH),
        "w1T": _wT_layout(inputs["w1"], DCH),
        "w2T": _wT_layout(inputs["w2"], FCH),
        "bv": np.ascontiguousarray(f32("bv")[None, :]).astype(NPBF16),
        "bq": _pcol(inputs["bq"], DCH),
        "bk": _pcol(inputs["bk"], DCH),
        "bo": _pcol(inputs["bo"], DCH),
        "b1": _pcol(inputs["b1"], FCH),
        "b2": _pcol(inputs["b2"], DCH),
        "g1": _pcol(inputs["g1"], DCH),
        "beta1": _pcol(inputs["beta1"], DCH),
        "g2": _pcol(inputs["g2"], DCH),
        "beta2": _pcol(inputs["beta2"], DCH),
    }
    x = f32("x")
    maps = []
    for core in range(NCORES):
        g, r = divmod(core, GROUP)
        xs = x[g, r * TOK : (r + 1) * TOK, :]  # [TOK, D]
        xT = xs.T.reshape(DCH, 128, TOK).transpose(1, 0, 2)
        m = dict(shared)
        m["xT"] = np.ascontiguousarray(xT).astype(NPBF16).reshape(
            128, DCH * TOK)
        maps.append(m)
    return maps


def _postprocess(res):
    shards = []
    for i in range(NCORES):
        o = np.asarray(res.results[i]["out_shard"], np.float32)
        y = o.reshape(128, DCH, TOK).transpose(2, 1, 0).reshape(TOK, D)
        shards.append(y)
    return np.concatenate(shards, axis=0).reshape(B, S, D).astype(np.float32)


def kernel(**inputs):
    nc = _build()
    maps = _in_maps(inputs)
    res = run_bass_kernel_spmd(nc, maps, core_ids=list(range(NCORES)))
    return _postprocess(res)
